# revision 1
# baseline (speedup 1.0000x reference)
"""Trainium2 Bass kernel for nn_MemoryEngineLayer (scatter_memory).

Contract: kernel(**inputs) takes FULL unsharded inputs (as produced by
setup_inputs()) and returns the FULL [B, T, H] output. Internally the batch
dim (B=8) is sharded across 8 NeuronCores (pure data parallelism); each core
runs the T=4096 recurrent scan for its own sequence.

Math (validated against the jax reference, rel err ~1e-8):
  m'_t   = x_t @ (beta/gamma * basis[:, :256])           # [256], top-8 inject
  inj'_t = where(|m'_t| >= kth8(|m'_t|), m'_t, 0)
  u_t    = r_{t-1} u_{t-1} + (g/gamma) r_{t-2} u_{t-2} + inj'_t   (complex 256)
  s_t    = 1 + 2 r_{t-1} <u_{t-1}, P_t> + ||P_t||^2,  P_t = g' r_{t-2} u_{t-2} + inj'_t
  r_t    = 1/sqrt(s_t + 1e-16)        # scale-invariant renorm
  y_t    = x_t + Re(r_t u_t) @ (alpha * bg * basis[:, :256]).T

On-chip layout: complex state as [128 partitions, 4 cols] = (re_lo, re_hi,
im_lo, im_hi) with slot s = q*128 + p. The per-step norm uses the identity
||r u_{t-1}|| = 1, keeping the partition_all_reduce off the r-dependency
chain.
"""

import numpy as np

H, MEM, S = 1024, 256, 272
B, T = 8, 4096
TOPK = 8
GAMMA, BETA, PTS = 0.92, 0.08, 0.4
PCH = 128  # timesteps per chunk

_program_cache = {}


def _sigmoid(v):
    return 1.0 / (1.0 + np.exp(-v.astype(np.float64)))


def _build_program(n_chunks: int, loop_reps: int = 1, pp_on_act: bool = False, fused_n: bool = True, z_on_dve: bool = False, gr_on_dve: bool = False, t_no_allred: bool = False, t_no_chain: bool = False, pe_allred: bool = True):
    import concourse.bass as bass
    import concourse.bacc as bacc
    import concourse.mybir as mybir
    from concourse.tile import TileContext
    from concourse.masks import make_identity
    from concourse import bass_isa

    f32 = mybir.dt.float32
    Alu = mybir.AluOpType
    Act = mybir.ActivationFunctionType
    Tq = n_chunks * PCH

    from concourse._compat import get_trn_type
    nc = bacc.Bacc(get_trn_type() or "TRN2", target_bir_lowering=False, debug=False)
    xb = nc.declare_dram_parameter("xb", [Tq, H], f32, isOutput=False)
    basis_m = nc.declare_dram_parameter("basis_m", [128, 8 * 256], f32, isOutput=False)
    basis_y = nc.declare_dram_parameter("basis_y", [128, 16 * 128], f32, isOutput=False)
    v0d = nc.declare_dram_parameter("v0", [128, 4], f32, isOutput=False)
    scal = nc.declare_dram_parameter("scal", [128, 3], f32, isOutput=False)  # [ones, gp, one]
    yb = nc.declare_dram_parameter("yb", [Tq, H], f32, isOutput=True)

    with TileContext(nc) as tc:
        with (
            tc.tile_pool(name="const", bufs=1) as cpool,
            tc.tile_pool(name="xio", bufs=4) as xpool,
            tc.tile_pool(name="work", bufs=2) as wpool,
            tc.tile_pool(name="scan", bufs=4) as spool,
            tc.tile_pool(name="ps_t", bufs=2, space="PSUM") as ps_t,
            tc.tile_pool(name="ps_m", bufs=1, space="PSUM") as ps_m,
            tc.tile_pool(name="ps_y", bufs=2, space="PSUM") as ps_y,
            tc.tile_pool(name="ps_r", bufs=2, space="PSUM") as ps_r,
        ):
            # ---- constants ----
            bm_sb = cpool.tile([128, 8 * 256], f32, tag="bm")
            nc.sync.dma_start(bm_sb, basis_m[:])
            by_sb = cpool.tile([128, 16 * 128], f32, tag="by")
            nc.sync.dma_start(by_sb, basis_y[:])
            v0_sb = cpool.tile([128, 4], f32, tag="v0")
            nc.sync.dma_start(v0_sb, v0d[:])
            sc_sb = cpool.tile([128, 3], f32, tag="sc")
            nc.sync.dma_start(sc_sb, scal[:])
            ident = cpool.tile([128, 128], f32, tag="ident")
            make_identity(nc, ident[:])
            ones_row = cpool.tile([1, 128], f32, tag="ones_row")
            nc.vector.memset(ones_row[:], 1.0)
            ones_mat = cpool.tile([128, 128], f32, tag="ones_mat")
            nc.vector.memset(ones_mat[:], 1.0)

            ones_ap = sc_sb[:, 0:1]
            gp_ap = sc_sb[:, 1:2]
            one_ap = sc_sb[:, 2:3]

            # Warm-ups: walrus allows at most ONE sync wait per PE matmul, so
            # make each engine observe the constant tiles via single-dep ops
            # before any real consumer needs them together.
            warm_acc = ps_m.tile([128, 1], f32, tag="warm")

            def pe_touch(sb_slice):
                # tiny N=1 matmul whose ONLY fresh dep is sb_slice; writes to
                # the shared warm PSUM tile (same-engine WAW, no semaphore)
                nc.tensor.matmul(
                    warm_acc[:], sb_slice, ident[:, 0:1], start=True, stop=True
                )

            pe_touch(ident[:, 0:128])  # waits Pool (make_identity) only
            pe_touch(bm_sb[:, 0:128])  # waits bm DMA only
            pe_touch(by_sb[:, 0:128])  # waits by DMA only
            wsc = spool.tile([128, 4], f32, tag="wsc")
            nc.vector.tensor_copy(wsc[:], v0_sb[:])  # DVE observes v0 DMA
            wsc2 = spool.tile([128, 3], f32, tag="wsc2")
            nc.vector.tensor_copy(wsc2[:], sc_sb[:])  # DVE observes scal DMA
            wsc3 = spool.tile([128, 3], f32, tag="wsc3")
            nc.scalar.copy(wsc3[:], sc_sb[:])  # ACT observes scal DMA

            # scan state carried across chunks (python vars reference tiles)
            u1, u2 = v0_sb, v0_sb          # u_{t-1}, u_{t-2}
            r1 = ones_ap                    # r_{t-1} [128,1]
            # FIFO of gp*r values: step t pops gp*r_{t-2}, pushes gp*r_t
            gr_fifo = [gp_ap, gp_ap]        # gp*r_{-2}, gp*r_{-1}

            # P ring: 4 persistent [128,5] tiles; cols 0:4 hold P_t, col 4 is
            # the constant sqrt(1/128) so ACT Square+accum yields
            # sum(P^2) + 1/128 per partition -> +1 total after the
            # partition_all_reduce (the "1" of s = 1 + 2 r IP + ||P||^2).
            P_ring = []
            for i in range(4):
                pring_tile = cpool.tile([128, 5], f32, tag=f"Pring{i}")
                P_ring.append(pring_tile)
                nc.vector.memset(pring_tile[:, 4:5], float(np.sqrt(1.0 / 128.0)))

            from contextlib import nullcontext
            rep_ctx = tc.For_i(0, loop_reps, 1) if loop_reps > 1 else nullcontext()
            with rep_ctx:
              # (re)bind scan state at body start so each repetition restarts
              u1, u2 = v0_sb, v0_sb
              r1 = ones_ap
              gr_fifo = [gp_ap, gp_ap]
              for c in range(n_chunks):
                  t0 = c * PCH
                  x_sb = xpool.tile([128, H], f32, tag="x")
                  nc.sync.dma_start(x_sb, xb[t0 : t0 + PCH, :])

                  # transpose x chunk -> xT [h, t] blocks
                  xT_sb = xpool.tile([128, H], f32, tag="xT")
                  # single-dep touch so the real transposes don't combine the
                  # x-DMA wait with a PSUM-slot-release wait
                  pe_touch(x_sb[:, 0:128])
                  for hi in range(8):
                      tps = ps_t.tile([128, 128], f32, tag="tps")
                      nc.tensor.transpose(tps, x_sb[:, hi * 128 : (hi + 1) * 128], ident[:])
                      nc.scalar.copy(xT_sb[:, hi * 128 : (hi + 1) * 128], tps[:])

                  # m' = x @ Wm  -> psum [t=128, s=256]
                  m_ps = ps_m.tile([128, 256], f32, tag="m")
                  for hi in range(8):
                      nc.tensor.matmul(
                          m_ps[:],
                          xT_sb[:, hi * 128 : (hi + 1) * 128],
                          bm_sb[:, hi * 256 : (hi + 1) * 256],
                          start=(hi == 0),
                          stop=(hi == 7),
                      )

                  # top-8 threshold + injection
                  mag = wpool.tile([128, 256], f32, tag="mag")
                  nc.scalar.activation(mag[:], m_ps[:], Act.Abs)
                  mx8 = wpool.tile([128, 8], f32, tag="mx8")
                  nc.vector.max(mx8[:], mag[:])
                  inj = wpool.tile([128, 256], f32, tag="inj")
                  nc.vector.scalar_tensor_tensor(
                      out=inj[:], in0=mag[:], scalar=mx8[:, 7:8], in1=m_ps[:],
                      op0=Alu.is_ge, op1=Alu.mult,
                  )

                  # transpose inj -> injT [slot, t] with zero upper half (imag)
                  injT = wpool.tile([128, 512], f32, tag="injT")
                  nc.gpsimd.memset(injT[:, 256:512], 0.0)
                  for q in range(2):
                      tps = ps_t.tile([128, 128], f32, tag="tps")
                      nc.tensor.transpose(tps, inj[:, q * 128 : (q + 1) * 128], ident[:])
                      nc.scalar.copy(injT[:, q * 128 : (q + 1) * 128], tps[:])

                  z_sb = wpool.tile([128, 256], f32, tag="z")

                  # ---- the sequential scan: 128 steps ----
                  for t in range(PCH):
                      inj4 = injT[:, t : 512 : 128]  # (re_lo, re_hi, 0, 0)
                      gr2 = gr_fifo.pop(0) if not t_no_chain else gp_ap
                      P = P_ring[t % 4]
                      nc.vector.scalar_tensor_tensor(
                          out=P[:, 0:4], in0=u2[:], scalar=gr2, in1=inj4,
                          op0=Alu.mult, op1=Alu.add,
                      )
                      u = spool.tile([128, 4], f32, tag="u")
                      nc.vector.scalar_tensor_tensor(
                          out=u[:], in0=u1[:], scalar=(ones_ap if t_no_chain else r1), in1=P[:, 0:4],
                          op0=Alu.mult, op1=Alu.add,
                      )
                      # sq2 col0 = sum(2*u1*P) (DVE), col1 = sum(P^2)+1/128 (ACT)
                      # (tensor_tensor_reduce crashes the device on this runtime;
                      # stt accum_out / ACT Square accum_out are the variants
                      # that work)
                      sq2 = spool.tile([128, 2], f32, tag="sq2")
                      d0 = spool.tile([128, 4], f32, tag="d0")
                      nc.vector.scalar_tensor_tensor(
                          out=d0[:], in0=u1[:], scalar=2.0, in1=P[:, 0:4],
                          op0=Alu.mult, op1=Alu.mult, accum_out=sq2[:, 0:1],
                      )
                      d1 = spool.tile([128, 5], f32, tag="d1")
                      if pp_on_act:
                          nc.scalar.activation(
                              d1[:], P[:], Act.Square, accum_out=sq2[:, 1:2]
                          )
                      else:
                          nc.vector.scalar_tensor_tensor(
                              out=d1[:], in0=P[:], scalar=1.0, in1=P[:],
                              op0=Alu.mult, op1=Alu.mult, accum_out=sq2[:, 1:2],
                          )
                      if pe_allred and not t_no_allred:
                          # one all-ones matmul = partition collapse AND
                          # broadcast (the Pool partition_all_reduce costs
                          # ~1us/step on HW)
                          sq2r = ps_r.tile([128, 2], f32, tag="ps_b")
                          nc.tensor.matmul(
                              sq2r[:], ones_mat[:], sq2[:], start=True, stop=True
                          )
                          # PP+1 must live in SBUF (stt can't read two PSUM
                          # operands; ACT bias must be SBUF)
                          pp1_sb = spool.tile([128, 1], f32, tag="pp1")
                          nc.scalar.copy(pp1_sb[:], sq2r[:, 1:2])
                      else:
                          sq2r = spool.tile([128, 2], f32, tag="sq2r")
                          if t_no_allred:  # timing-only: wrong math
                              nc.vector.tensor_copy(sq2r[:], sq2[:])
                          else:
                              nc.gpsimd.partition_all_reduce(
                                  sq2r[:], sq2[:], 128, bass_isa.ReduceOp.add
                              )
                      n_t = spool.tile([128, 1], f32, tag="n")
                      if t_no_chain:  # timing-only: wrong math, breaks r recurrence
                          s_t = spool.tile([128, 1], f32, tag="s")
                          nc.vector.scalar_tensor_tensor(
                              out=s_t[:], in0=sq2r[:, 0:1], scalar=ones_ap, in1=sq2r[:, 1:2],
                              op0=Alu.mult, op1=Alu.add,
                          )
                          nc.scalar.activation(n_t[:], s_t[:], Act.Sqrt)
                      elif fused_n and pe_allred:
                          # n = sqrt(2*IP_tot*r1 + (PP_tot+1)) in one ACT op,
                          # reading 2IP from PSUM and PP+1 from SBUF
                          nc.scalar.activation(
                              n_t[:], sq2r[:, 0:1], Act.Sqrt, scale=r1, bias=pp1_sb[:]
                          )
                      elif fused_n:
                          # n = sqrt(2*IP*r1 + (||P||^2 + 1)) in one ACT op
                          nc.scalar.activation(
                              n_t[:], sq2r[:, 0:1], Act.Sqrt, scale=r1, bias=sq2r[:, 1:2]
                          )
                      else:
                          s_t = spool.tile([128, 1], f32, tag="s")
                          nc.vector.scalar_tensor_tensor(
                              out=s_t[:], in0=sq2r[:, 0:1], scalar=r1, in1=sq2r[:, 1:2],
                              op0=Alu.mult, op1=Alu.add,
                          )
                          nc.scalar.activation(n_t[:], s_t[:], Act.Sqrt)
                      r_t = spool.tile([128, 1], f32, tag="r")
                      nc.vector.reciprocal(r_t[:], n_t[:])
                      gr_t = spool.tile([128, 1], f32, tag="gr")
                      if gr_on_dve:
                          nc.vector.tensor_scalar_mul(gr_t[:], r_t[:], gp_ap)
                      else:
                          nc.scalar.activation(gr_t[:], r_t[:], Act.Copy, scale=gp_ap)
                      # z_t = Re(r_t * u_t)
                      if z_on_dve:
                          nc.vector.tensor_scalar_mul(
                              z_sb[:, t : 256 : 128], u[:, 0:2], r_t[:]
                          )
                      else:
                          nc.scalar.activation(
                              z_sb[:, t : 256 : 128], u[:, 0:2], Act.Copy, scale=r_t[:]
                          )
                      u2, u1 = u1, u
                      gr_fifo.append(gr_t[:])
                      r1 = r_t[:]

                  # ---- y^T = Wy^T @ z + x^T ; transpose back; store ----
                  y_sb = xpool.tile([128, H], f32, tag="y")
                  for hi in range(8):
                      yT_ps = ps_y.tile([128, 128], f32, tag="yT")
                      nc.tensor.matmul(
                          yT_ps[:], by_sb[:, (hi * 2 + 0) * 128 : (hi * 2 + 1) * 128],
                          z_sb[:, 0:128], start=True, stop=False,
                      )
                      nc.tensor.matmul(
                          yT_ps[:], by_sb[:, (hi * 2 + 1) * 128 : (hi * 2 + 2) * 128],
                          z_sb[:, 128:256], start=False, stop=False,
                      )
                      nc.tensor.matmul(
                          yT_ps[:], ident[:], xT_sb[:, hi * 128 : (hi + 1) * 128],
                          start=False, stop=True,
                      )
                      yT_sb = wpool.tile([128, 128], f32, tag="yTs")
                      nc.scalar.copy(yT_sb[:], yT_ps[:])
                      y_ps = ps_y.tile([128, 128], f32, tag="yT")
                      nc.tensor.transpose(y_ps, yT_sb[:], ident[:])
                      nc.scalar.copy(y_sb[:, hi * 128 : (hi + 1) * 128], y_ps[:])
                  nc.sync.dma_start(yb[t0 : t0 + PCH, :], y_sb[:])

    nc.compile()
    return nc


def _host_pack(inputs):
    """Fold all small parameters host-side; returns per-core constant arrays."""
    basis = np.asarray(inputs["basis"], np.float32)
    alpha = float(np.asarray(inputs["alpha"]))
    w_r = np.asarray(inputs["w_r"], np.float32)
    bg = _sigmoid(np.asarray(inputs["breadth_gate"], np.float32))

    g = _sigmoid(w_r)
    assert np.all(g[:MEM] == g[0]), "vector w_r gate not supported by fast path"
    gp = float(g[0]) / GAMMA

    Wm = (basis[:, :MEM] * (BETA / GAMMA)).astype(np.float32)  # [H, 256]
    Wy = (basis[:, :MEM] * (alpha * bg[None, :MEM])).astype(np.float32)

    # basis_m blocks: block hi = Wm[hi*128:(hi+1)*128, :]  -> cols [hi*256, ...)
    basis_m = np.concatenate(
        [Wm[hi * 128 : (hi + 1) * 128, :] for hi in range(8)], axis=1
    ).astype(np.float32)  # [128, 2048]
    WyT = np.ascontiguousarray(Wy.T)  # [256, 1024]
    blocks = []
    for hi in range(8):
        for q in range(2):
            blocks.append(WyT[q * 128 : (q + 1) * 128, hi * 128 : (hi + 1) * 128])
    basis_y = np.concatenate(blocks, axis=1).astype(np.float32)  # [128, 2048]

    t0c = (
        np.asarray(inputs["tape_init_re"], np.float32)
        + 1j * np.asarray(inputs["tape_init_im"], np.float32)
    )[:MEM].astype(np.complex64)
    nrm = np.float32(np.sqrt(max(float((np.abs(t0c) ** 2).sum(dtype=np.float32)), 1e-16)))
    v0c = (t0c / nrm).astype(np.complex64)
    v0 = np.stack(
        [v0c.real[:128], v0c.real[128:], v0c.imag[:128], v0c.imag[128:]], axis=1
    ).astype(np.float32)  # [128, 4]

    scal = np.empty((128, 3), np.float32)
    scal[:, 0] = 1.0
    scal[:, 1] = gp
    scal[:, 2] = 1.0
    return basis_m, basis_y, v0, scal


def _fast_path_ok(inputs):
    z = lambda k: np.all(np.asarray(inputs[k]) == 0)
    g = _sigmoid(np.asarray(inputs["w_r"], np.float32))
    return (
        z("torque_rotation")
        and z("epsilon_scale")
        and z("epsilon_diag")
        and z("pred_scale")
        and z("pred_diag")
        and bool(np.all(g[:MEM] == g[0]))
    )


def _numpy_fallback(inputs):
    """General-case reference implementation (host). Only used if the inputs
    violate the fast-path structure (never the case for this problem's
    generator); keeps kernel() total."""
    import jax

    with jax.default_device(jax.devices("cpu")[0]):
        import jax.numpy as jnp
        from jax import lax

        x = jnp.asarray(inputs["x"])
        basis = jnp.asarray(inputs["basis"])
        active = jnp.arange(S) < MEM
        amf = active.astype(jnp.float32)
        eta = jax.nn.softplus(jnp.asarray(inputs["eta_raw"]))
        eps = (jnp.asarray(inputs["epsilon_factor"]) * jnp.asarray(inputs["epsilon_scale"])) @ jnp.asarray(
            inputs["epsilon_factor"]).T + jnp.diag(jnp.asarray(inputs["epsilon_diag"]))
        wp = (jnp.asarray(inputs["pred_factor"]) * jnp.asarray(inputs["pred_scale"])) @ jnp.asarray(
            inputs["pred_factor"]).T + jnp.diag(jnp.asarray(inputs["pred_diag"]))
        eps_c = eps.astype(jnp.complex64)
        wp_c = wp.astype(jnp.complex64)
        rot = jnp.exp(1j * jnp.asarray(inputs["torque_rotation"]).astype(jnp.complex64))
        wr_gate = jax.nn.sigmoid(jnp.asarray(inputs["w_r"]))
        bg = jax.nn.sigmoid(jnp.asarray(inputs["breadth_gate"]))
        alpha = jnp.asarray(inputs["alpha"])

        def renorm(tape):
            masked = tape * amf
            nrm = jnp.sqrt(jnp.maximum((jnp.abs(masked) ** 2).sum(-1, keepdims=True), 1e-16))
            return masked / nrm

        tape0 = (jnp.asarray(inputs["tape_init_re"]) + 1j * jnp.asarray(inputs["tape_init_im"])) * amf
        tape0 = renorm(jnp.broadcast_to(tape0, (B, S)))

        def step(carry, x_t):
            tape, prev = carry
            m = jnp.einsum("hs,bh->bs", basis, x_t)
            mag = jnp.abs(m) * amf
            kth = lax.top_k(mag, TOPK)[0][:, -1:]
            injv = jnp.where((mag >= kth) & active, m, 0.0).astype(jnp.complex64)
            rotated = tape * rot
            drive = jnp.einsum("st,bt->bs", eps_c, rotated)
            pred = jnp.einsum("st,bt->bs", wp_c, rotated)
            new = (GAMMA * rotated + eta * drive + BETA * injv + PTS * 1j * pred + wr_gate * prev)
            new = renorm(new)
            y = x_t + alpha * jnp.einsum("hs,bs->bh", basis, bg * new.real)
            return (new, tape), y

        (_, _), ys = lax.scan(step, (tape0, tape0), jnp.swapaxes(x, 0, 1))
        return np.asarray(jnp.swapaxes(ys, 0, 1))


def kernel(n_chunks: int = T // PCH, _want_trace: bool = False, **inputs) -> np.ndarray:
    from concourse.bass_utils import run_bass_kernel_spmd

    x = np.ascontiguousarray(np.asarray(inputs["x"], np.float32))
    assert x.shape == (B, T, H)

    if not _fast_path_ok(inputs):
        return _numpy_fallback(inputs)

    basis_m, basis_y, v0, scal = _host_pack(inputs)

    key = n_chunks
    if key not in _program_cache:
        _program_cache[key] = _build_program(n_chunks)
    nc = _program_cache[key]

    Tq = n_chunks * PCH
    core_ids = list(range(B))
    in_maps = [
        {
            "xb": np.ascontiguousarray(x[b, :Tq]),
            "basis_m": basis_m,
            "basis_y": basis_y,
            "v0": v0,
            "scal": scal,
        }
        for b in core_ids
    ]
    res = run_bass_kernel_spmd(nc, in_maps, core_ids, trace=_want_trace)
    out = np.empty((B, Tq, H), np.float32)
    for b in core_ids:
        out[b] = res.results[b]["yb"]
    if _want_trace:
        kernel._last_results = res
    return out



# revision 23
# speedup vs baseline: 4.7472x; 4.7472x over previous
"""Trainium2 Bass kernel for nn_MemoryEngineLayer (scatter_memory).

Contract: kernel(**inputs) takes FULL unsharded inputs (as produced by
setup_inputs()) and returns the FULL [B, T, H] output. Internally the batch
dim (B=8) is sharded across 8 NeuronCores (pure data parallelism); each core
runs the T=4096 recurrent scan for its own sequence.

Math (validated against the jax reference):
  m_t    = x_t @ (beta * basis[:, :256])          # [256], top-8 inject (binj)
  w_t    = a_t w_{t-1} + b_t w_{t-2} + binj_t     # w_t = pre-norm tape vector
  n_t^2  = ||w_t||^2
  a_{t+1} = gamma / n_t ;  b_{t+1} = (g/gamma) * a_t   # g = sigmoid(w_r)
  y_t    = x_t + Re(w_t)/n_t @ (alpha*bg*basis[:, :256]).T

Two scan implementations (default root="pipe"; root="ars" is the simpler
fallback):
  "ars":  per-step chain DVE(P,w,w*w-accum) -> PE ones-matmul (partition
          all-reduce+broadcast) -> ACT Abs_reciprocal_sqrt -> next step.
          ~2.05 us/step (the ACT round trip costs ~1.1 us).
  "pipe": software-pipelined — slot t's DVE builds w_t and u_{t+1} plus the
          three norm products (||w_t||^2, 2<w_t,u_{t+1}>, ||u_{t+1}||^2);
          the ACT chain one slot later assembles
            n_t^2 = a_t^2 q1 + a_t (2 q2) + q3   (= ||a_t w_{t-1} + u_t||^2)
          and emits a_{t+1} via Abs_reciprocal_sqrt with fused scale/bias.
          All cross-engine handoffs are one-way (DVE->PE->ACT->DVE with a
          slot of slack); ~1.55 us/step. Injection prep for chunk c+1 is
          emitted before chunk c's scan so the last slot is uniform.

On-chip layout: complex state as [128 partitions, 4 cols] = (re_lo, re_hi,
im_lo, im_hi) with slot s = q*128 + p. z output is stored UNNORMALIZED
(raw Re(w_t)); the 1/n_t scale is applied per-column to the y^T matmul
result using the broadcast a-row (a_{t+1} = gamma/n_t), with 1/gamma
folded into the y basis weights host-side.
"""

import numpy as np

H, MEM, S = 1024, 256, 272
B, T = 8, 4096
TOPK = 8
GAMMA, BETA, PTS = 0.92, 0.08, 0.4
PCH = 128  # timesteps per chunk

_program_cache = {}


def _sigmoid(v):
    return 1.0 / (1.0 + np.exp(-v.astype(np.float64)))


def _build_program(n_chunks: int, loop_reps: int = 1, root: str = "ars",
                   newton_iters: int = 2):
    if root == "pipe":
        return _build_program_pipe(n_chunks, loop_reps)
    import concourse.bass as bass
    import concourse.bacc as bacc
    import concourse.mybir as mybir
    from concourse.tile import TileContext
    from concourse.masks import make_identity

    f32 = mybir.dt.float32
    Alu = mybir.AluOpType
    Act = mybir.ActivationFunctionType
    Tq = n_chunks * PCH
    IG2 = 1.0 / GAMMA**2

    from concourse._compat import get_trn_type
    nc = bacc.Bacc(get_trn_type() or "TRN2", target_bir_lowering=False, debug=False)
    xb = nc.declare_dram_parameter("xb", [Tq, H], f32, isOutput=False)
    basis_m = nc.declare_dram_parameter("basis_m", [128, 8 * 256], f32, isOutput=False)
    basis_y = nc.declare_dram_parameter("basis_y", [128, 16 * 128], f32, isOutput=False)
    v0d = nc.declare_dram_parameter("v0", [128, 4], f32, isOutput=False)
    scal = nc.declare_dram_parameter("scal", [128, 3], f32, isOutput=False)  # [a0, b0, g/gamma]
    yb = nc.declare_dram_parameter("yb", [Tq, H], f32, isOutput=True)

    with TileContext(nc) as tc:
        with (
            tc.tile_pool(name="const", bufs=1) as cpool,
            tc.tile_pool(name="xio", bufs=4) as xpool,
            tc.tile_pool(name="work", bufs=2) as wpool,
            tc.tile_pool(name="scan", bufs=4) as spool,
            tc.tile_pool(name="zrow", bufs=2) as zpool,
            tc.tile_pool(name="ps_t", bufs=2, space="PSUM") as ps_t,
            tc.tile_pool(name="ps_m", bufs=1, space="PSUM") as ps_m,
            tc.tile_pool(name="ps_y", bufs=2, space="PSUM") as ps_y,
            tc.tile_pool(name="ps_n", bufs=2, space="PSUM") as ps_n,
        ):
            # ---- constants ----
            bm_sb = cpool.tile([128, 8 * 256], f32, tag="bm")
            nc.sync.dma_start(bm_sb, basis_m[:])
            by_sb = cpool.tile([128, 16 * 128], f32, tag="by")
            nc.sync.dma_start(by_sb, basis_y[:])
            v0_sb = cpool.tile([128, 4], f32, tag="v0")
            nc.sync.dma_start(v0_sb, v0d[:])
            sc_sb = cpool.tile([128, 3], f32, tag="sc")
            nc.sync.dma_start(sc_sb, scal[:])
            ident = cpool.tile([128, 128], f32, tag="ident")
            make_identity(nc, ident[:])
            ones_mat = cpool.tile([128, 128], f32, tag="ones_mat")
            nc.vector.memset(ones_mat[:], 1.0)
            gq_ap = sc_sb[:, 2:3]  # g/gamma

            # Warm-ups: walrus allows at most ONE sync wait per PE matmul, so
            # make each engine observe the constant tiles via single-dep ops
            # before any real consumer needs them together.
            warm_acc = ps_m.tile([128, 1], f32, tag="warm")

            def pe_touch(sb_slice):
                nc.tensor.matmul(
                    warm_acc[:], sb_slice, ident[:, 0:1], start=True, stop=True
                )

            pe_touch(ident[:, 0:128])      # waits Pool (make_identity) only
            pe_touch(ones_mat[:, 0:128])   # waits DVE memset only
            pe_touch(bm_sb[:, 0:128])      # waits bm DMA only
            pe_touch(by_sb[:, 0:128])      # waits by DMA only
            wsc = spool.tile([128, 4], f32, tag="wsc")
            nc.vector.tensor_copy(wsc[:], v0_sb[:])   # DVE observes v0 DMA
            wsc2 = spool.tile([128, 3], f32, tag="wsc2")
            nc.vector.tensor_copy(wsc2[:], sc_sb[:])  # DVE observes scal DMA
            wsc3 = spool.tile([128, 3], f32, tag="wsc3")
            nc.scalar.copy(wsc3[:], sc_sb[:])         # ACT observes scal DMA

            from contextlib import nullcontext
            rep_ctx = tc.For_i(0, loop_reps, 1) if loop_reps > 1 else nullcontext()
            with rep_ctx:
              # (re)bind scan state at body start so each repetition restarts
              w1, w2 = v0_sb, v0_sb
              a_ap = sc_sb[:, 0:1]
              b_ap = sc_sb[:, 1:2]
              for c in range(n_chunks):
                  t0 = c * PCH
                  x_sb = xpool.tile([128, H], f32, tag="x")
                  nc.sync.dma_start(x_sb, xb[t0 : t0 + PCH, :])

                  # transpose x chunk -> xT [h, t] blocks (for the m matmul)
                  xT_sb = xpool.tile([128, H], f32, tag="xT")
                  pe_touch(x_sb[:, 0:128])
                  for hi in range(8):
                      tps = ps_t.tile([128, 128], f32, tag="tps")
                      nc.tensor.transpose(tps, x_sb[:, hi * 128 : (hi + 1) * 128], ident[:])
                      nc.scalar.copy(xT_sb[:, hi * 128 : (hi + 1) * 128], tps[:])

                  # m = x @ (beta*basis)  -> psum [t=128, s=256]
                  m_ps = ps_m.tile([128, 256], f32, tag="m")
                  for hi in range(8):
                      nc.tensor.matmul(
                          m_ps[:],
                          xT_sb[:, hi * 128 : (hi + 1) * 128],
                          bm_sb[:, hi * 256 : (hi + 1) * 256],
                          start=(hi == 0),
                          stop=(hi == 7),
                      )

                  # top-8 threshold + injection (pre-scaled by beta)
                  mag = wpool.tile([128, 256], f32, tag="mag")
                  nc.scalar.activation(mag[:], m_ps[:], Act.Abs)
                  mx8 = wpool.tile([128, 8], f32, tag="mx8")
                  nc.vector.max(mx8[:], mag[:])
                  inj = wpool.tile([128, 256], f32, tag="inj")
                  nc.vector.scalar_tensor_tensor(
                      out=inj[:], in0=mag[:], scalar=mx8[:, 7:8], in1=m_ps[:],
                      op0=Alu.is_ge, op1=Alu.mult,
                  )

                  # transpose inj -> binjT [slot, t] with zero upper half (imag)
                  binjT = wpool.tile([128, 512], f32, tag="binjT")
                  nc.gpsimd.memset(binjT[:, 256:512], 0.0)
                  for q in range(2):
                      tps = ps_t.tile([128, 128], f32, tag="tps")
                      nc.tensor.transpose(tps, inj[:, q * 128 : (q + 1) * 128], ident[:])
                      nc.scalar.copy(binjT[:, q * 128 : (q + 1) * 128], tps[:])

                  zc = zpool.tile([128, 256], f32, tag="zc")
                  arow = zpool.tile([128, 129], f32, tag="arow")

                  # ---- the sequential scan: 128 steps ----
                  for t in range(PCH):
                      binj4 = binjT[:, t : 512 : 128]  # (re_lo, re_hi, 0, 0)
                      P = spool.tile([128, 4], f32, tag="P")
                      nc.vector.scalar_tensor_tensor(
                          out=P[:], in0=w2[:], scalar=b_ap, in1=binj4,
                          op0=Alu.mult, op1=Alu.add,
                      )
                      w = spool.tile([128, 4], f32, tag="w")
                      nc.vector.scalar_tensor_tensor(
                          out=w[:], in0=w1[:], scalar=a_ap, in1=P[:],
                          op0=Alu.mult, op1=Alu.add,
                      )
                      npart = spool.tile([128, 1], f32, tag="np")
                      dtr = spool.tile([128, 4], f32, tag="d")
                      nc.vector.scalar_tensor_tensor(
                          out=dtr[:], in0=w[:], scalar=1.0, in1=w[:],
                          op0=Alu.mult, op1=Alu.mult, accum_out=npart[:],
                      )
                      nps = ps_n.tile([128, 1], f32, tag="nps")
                      nc.tensor.matmul(nps[:], ones_mat[:], npart[:], start=True, stop=True)
                      a_new = arow[:, t + 1 : t + 2]
                      if root == "ars":
                          # a = gamma * rsqrt(n^2) in one ACT op
                          nc.scalar.activation(a_new, nps[:], Act.Abs_reciprocal_sqrt,
                                               scale=IG2)
                      elif root == "sqrtr":
                          nsb = spool.tile([128, 1], f32, tag="ns")
                          nc.scalar.activation(nsb[:], nps[:], Act.Sqrt, scale=IG2)
                          nc.vector.reciprocal(a_new, nsb[:])
                      elif root in ("newton1", "newton2"):
                          n_it = 1 if root == "newton1" else newton_iters
                          cur = a_ap
                          for it in range(n_it):
                              hh = spool.tile([128, 1], f32, tag="h")
                              nc.vector.scalar_tensor_tensor(
                                  out=hh[:], in0=cur, scalar=cur, in1=nps[:],
                                  op0=Alu.mult, op1=Alu.mult,
                              )
                              tt = spool.tile([128, 1], f32, tag="t2")
                              nc.vector.tensor_scalar(
                                  out=tt[:], in0=hh[:], scalar1=-0.5 * IG2,
                                  scalar2=1.5, op0=Alu.mult, op1=Alu.add,
                              )
                              outp = a_new if it == n_it - 1 else spool.tile(
                                  [128, 1], f32, tag="anx")
                              nc.vector.tensor_scalar(
                                  out=outp, in0=tt[:], scalar1=cur, scalar2=None,
                                  op0=Alu.mult,
                              )
                              cur = outp
                      else:
                          raise ValueError(root)
                      # off-chain: b for step t+1; z output on ACT (raw Re(w))
                      bnew = spool.tile([128, 1], f32, tag="bn")
                      nc.vector.tensor_scalar(
                          out=bnew[:], in0=a_ap, scalar1=gq_ap, scalar2=None,
                          op0=Alu.mult,
                      )
                      nc.scalar.activation(zc[:, t : 256 : 128], w[:, 0:2], Act.Copy)
                      w2, w1 = w1, w
                      a_ap = a_new
                      b_ap = bnew[:]

                  # ---- y^T = Wy^T @ z, scale cols by a-row, transpose, +x ----
                  y_sb = xpool.tile([128, H], f32, tag="y")
                  for hi in range(8):
                      yT_ps = ps_y.tile([128, 128], f32, tag="yT")
                      nc.tensor.matmul(
                          yT_ps[:], by_sb[:, (hi * 2 + 0) * 128 : (hi * 2 + 1) * 128],
                          zc[:, 0:128], start=True, stop=False,
                      )
                      nc.tensor.matmul(
                          yT_ps[:], by_sb[:, (hi * 2 + 1) * 128 : (hi * 2 + 2) * 128],
                          zc[:, 128:256], start=False, stop=True,
                      )
                      yTs = wpool.tile([128, 128], f32, tag="yTs")
                      nc.vector.scalar_tensor_tensor(
                          out=yTs[:], in0=yT_ps[:], scalar=1.0, in1=arow[:, 1:129],
                          op0=Alu.mult, op1=Alu.mult,
                      )
                      y_ps = ps_y.tile([128, 128], f32, tag="yT")
                      nc.tensor.matmul(y_ps[:], yTs[:], ident[:], start=True, stop=False)
                      nc.tensor.matmul(
                          y_ps[:], ident[:], x_sb[:, hi * 128 : (hi + 1) * 128],
                          start=False, stop=True,
                      )
                      nc.scalar.copy(y_sb[:, hi * 128 : (hi + 1) * 128], y_ps[:])
                  nc.sync.dma_start(yb[t0 : t0 + PCH, :], y_sb[:])

    nc.compile()
    return nc


def _build_program_pipe(n_chunks: int, loop_reps: int = 1, Z_ENGINE: str = "dve",
                        psn_bufs: int = 2, fuse_z: bool = False):
    """Software-pipelined scan: the root chain lives entirely on ACT with
    one-way engine handoffs (no per-step cross-engine round trip).

    Slot t computes (DVE) w_t = a_t*w_{t-1} + u_t, u_{t+1} = b_{t+1}*w_{t-1}
    + binj_{t+1}, and the three norm products (||w_t||^2, 2<w_t,u_{t+1}>,
    ||u_{t+1}||^2) -> PE all-reduce; the ACT chain at slot t consumes slot
    t-1's reduced products plus its own a_t to produce a_{t+1} via
      n_t^2 = a_t^2 q1 + a_t (2 q2) + q3  (== ||a_t w_{t-1} + u_t||^2)
      a_{t+1} = gamma * rsqrt(n_t^2)  [Abs_reciprocal_sqrt, scale/bias fused]
    z goes to GPSIMD. Chunk c+1's injection prep is emitted before chunk c's
    scan so the last slot can build u for the next chunk uniformly.
    """
    import concourse.bass as bass
    import concourse.bacc as bacc
    import concourse.mybir as mybir
    from concourse.tile import TileContext
    from concourse.masks import make_identity

    f32 = mybir.dt.float32
    Alu = mybir.AluOpType
    Act = mybir.ActivationFunctionType
    Tq = n_chunks * PCH
    IG2 = 1.0 / GAMMA**2

    from concourse._compat import get_trn_type
    nc = bacc.Bacc(get_trn_type() or "TRN2", target_bir_lowering=False, debug=False)
    xb = nc.declare_dram_parameter("xb", [Tq, H], f32, isOutput=False)
    basis_m = nc.declare_dram_parameter("basis_m", [128, 8 * 256], f32, isOutput=False)
    basis_y = nc.declare_dram_parameter("basis_y", [128, 16 * 128], f32, isOutput=False)
    v0d = nc.declare_dram_parameter("v0", [128, 4], f32, isOutput=False)
    scal = nc.declare_dram_parameter("scal", [128, 3], f32, isOutput=False)
    yb = nc.declare_dram_parameter("yb", [Tq, H], f32, isOutput=True)

    with TileContext(nc) as tc:
        with (
            tc.tile_pool(name="const", bufs=1) as cpool,
            tc.tile_pool(name="xio", bufs=4) as xpool,
            tc.tile_pool(name="work", bufs=2) as wpool,
            tc.tile_pool(name="scan", bufs=4) as spool,
            tc.tile_pool(name="zrow", bufs=2) as zpool,
            tc.tile_pool(name="ps_t", bufs=2, space="PSUM") as ps_t,
            tc.tile_pool(name="ps_m", bufs=1, space="PSUM") as ps_m,
            tc.tile_pool(name="ps_y", bufs=2, space="PSUM") as ps_y,
            tc.tile_pool(name="ps_n", bufs=psn_bufs, space="PSUM") as ps_n,
        ):
            bm_sb = cpool.tile([128, 8 * 256], f32, tag="bm")
            nc.sync.dma_start(bm_sb, basis_m[:])
            by_sb = cpool.tile([128, 16 * 128], f32, tag="by")
            nc.sync.dma_start(by_sb, basis_y[:])
            v0_sb = cpool.tile([128, 4], f32, tag="v0")
            nc.sync.dma_start(v0_sb, v0d[:])
            sc_sb = cpool.tile([128, 3], f32, tag="sc")
            nc.sync.dma_start(sc_sb, scal[:])
            ident = cpool.tile([128, 128], f32, tag="ident")
            make_identity(nc, ident[:])
            ones_mat = cpool.tile([128, 128], f32, tag="ones_mat")
            nc.vector.memset(ones_mat[:], 1.0)
            gq_ap = sc_sb[:, 2:3]

            warm_acc = ps_m.tile([128, 1], f32, tag="warm")

            def pe_touch(sb_slice):
                nc.tensor.matmul(
                    warm_acc[:], sb_slice, ident[:, 0:1], start=True, stop=True
                )

            pe_touch(ident[:, 0:128])
            pe_touch(ones_mat[:, 0:128])
            pe_touch(bm_sb[:, 0:128])
            pe_touch(by_sb[:, 0:128])
            wsc = spool.tile([128, 4], f32, tag="wsc")
            nc.vector.tensor_copy(wsc[:], v0_sb[:])
            wsc2 = spool.tile([128, 3], f32, tag="wsc2")
            nc.vector.tensor_copy(wsc2[:], sc_sb[:])
            wsc3 = spool.tile([128, 3], f32, tag="wsc3")
            nc.scalar.copy(wsc3[:], sc_sb[:])
            wsc4 = spool.tile([128, 3], f32, tag="wsc4")
            nc.gpsimd.tensor_copy(wsc4[:], sc_sb[:])  # Pool observes scal DMA

            def prep(c):
                """DMA + transpose + m-matmul + top-8 injection for chunk c."""
                t0 = c * PCH
                x_sb = xpool.tile([128, H], f32, tag="x")
                nc.sync.dma_start(x_sb, xb[t0 : t0 + PCH, :])
                xT_sb = xpool.tile([128, H], f32, tag="xT")
                pe_touch(x_sb[:, 0:128])
                for hi in range(8):
                    tps = ps_t.tile([128, 128], f32, tag="tps")
                    nc.tensor.transpose(tps, x_sb[:, hi * 128 : (hi + 1) * 128], ident[:])
                    nc.scalar.copy(xT_sb[:, hi * 128 : (hi + 1) * 128], tps[:])
                m_ps = ps_m.tile([128, 256], f32, tag="m")
                for hi in range(8):
                    nc.tensor.matmul(
                        m_ps[:],
                        xT_sb[:, hi * 128 : (hi + 1) * 128],
                        bm_sb[:, hi * 256 : (hi + 1) * 256],
                        start=(hi == 0),
                        stop=(hi == 7),
                    )
                mag = wpool.tile([128, 256], f32, tag="mag")
                nc.scalar.activation(mag[:], m_ps[:], Act.Abs)
                mx8 = wpool.tile([128, 8], f32, tag="mx8")
                nc.vector.max(mx8[:], mag[:])
                inj = wpool.tile([128, 256], f32, tag="inj")
                nc.vector.scalar_tensor_tensor(
                    out=inj[:], in0=mag[:], scalar=mx8[:, 7:8], in1=m_ps[:],
                    op0=Alu.is_ge, op1=Alu.mult,
                )
                binjT = wpool.tile([128, 512], f32, tag="binjT")
                nc.gpsimd.memset(binjT[:, 256:512], 0.0)
                for q in range(2):
                    tps = ps_t.tile([128, 128], f32, tag="tps")
                    nc.tensor.transpose(tps, inj[:, q * 128 : (q + 1) * 128], ident[:])
                    nc.scalar.copy(binjT[:, q * 128 : (q + 1) * 128], tps[:])
                return x_sb, binjT

            from contextlib import nullcontext
            rep_ctx = tc.For_i(0, loop_reps, 1) if loop_reps > 1 else nullcontext()
            with rep_ctx:
              w1 = v0_sb[:]              # w_{t-1} (AP)
              a_ap = sc_sb[:, 0:1]       # a_t
              x_cur, binjT_cur = prep(0)
              # bootstrap "slot -1": u_0 and its norm products
              u_cur = spool.tile([128, 4], f32, tag="u")
              nc.vector.scalar_tensor_tensor(
                  out=u_cur[:], in0=v0_sb[:], scalar=sc_sb[:, 1:2],
                  in1=binjT_cur[:, 0:512:128], op0=Alu.mult, op1=Alu.add,
              )
              qp = spool.tile([128, 3], f32, tag="qp")
              dtr = spool.tile([128, 4], f32, tag="d")
              nc.vector.scalar_tensor_tensor(
                  out=dtr[:], in0=v0_sb[:], scalar=1.0, in1=v0_sb[:],
                  op0=Alu.mult, op1=Alu.mult, accum_out=qp[:, 0:1],
              )
              dtr = spool.tile([128, 4], f32, tag="d")
              nc.vector.scalar_tensor_tensor(
                  out=dtr[:], in0=v0_sb[:], scalar=2.0, in1=u_cur[:],
                  op0=Alu.mult, op1=Alu.mult, accum_out=qp[:, 1:2],
              )
              dtr = spool.tile([128, 4], f32, tag="d")
              nc.vector.scalar_tensor_tensor(
                  out=dtr[:], in0=u_cur[:], scalar=1.0, in1=u_cur[:],
                  op0=Alu.mult, op1=Alu.mult, accum_out=qp[:, 2:3],
              )
              nq_prev = ps_n.tile([128, 3], f32, tag="nq")
              nc.tensor.matmul(nq_prev[:], ones_mat[:], qp[:], start=True, stop=True)

              for c in range(n_chunks):
                  if c + 1 < n_chunks:
                      x_next, binjT_next = prep(c + 1)
                  else:
                      x_next, binjT_next = None, None
                  zc = zpool.tile([128, 512 if fuse_z else 256], f32, tag="zc")
                  arow = zpool.tile([128, 129], f32, tag="arow")

                  for t in range(PCH):
                      last_slot = (t == PCH - 1) and (binjT_next is None)
                      # ---- ACT root chain: a_{t+1} from nq_prev + a_t ----
                      qsb = spool.tile([128, 3], f32, tag="qsb")
                      nc.scalar.activation(qsb[:], nq_prev[:], Act.Copy, scale=IG2)
                      h2 = spool.tile([128, 1], f32, tag="h2")
                      nc.scalar.activation(h2[:], qsb[:, 0:1], Act.Identity,
                                           scale=a_ap, bias=qsb[:, 1:2])
                      a_new = arow[:, t + 1 : t + 2]
                      nc.scalar.activation(a_new, h2[:], Act.Abs_reciprocal_sqrt,
                                           scale=a_ap, bias=qsb[:, 2:3])
                      # ---- DVE: state build + norm products ----
                      if fuse_z:
                          # w_t lives in the z-chunk tile as a stride-128
                          # 4-col slice (block layout: re_lo | re_hi | im_lo |
                          # im_hi); y matmuls then read re blocks contiguous.
                          w = zc[:, t : 512 : 128]
                      else:
                          wt = spool.tile([128, 4], f32, tag="w")
                          w = wt[:]
                      nc.vector.scalar_tensor_tensor(
                          out=w, in0=w1, scalar=a_ap, in1=u_cur[:],
                          op0=Alu.mult, op1=Alu.add,
                      )
                      if not last_slot:
                          bnew = spool.tile([128, 1], f32, tag="bn")
                          nc.vector.tensor_scalar(
                              out=bnew[:], in0=a_ap, scalar1=gq_ap, scalar2=None,
                              op0=Alu.mult,
                          )
                          bt_src = binjT_cur if t < PCH - 1 else binjT_next
                          tt = (t + 1) % PCH
                          u_next = spool.tile([128, 4], f32, tag="u")
                          nc.vector.scalar_tensor_tensor(
                              out=u_next[:], in0=w1, scalar=bnew[:],
                              in1=bt_src[:, tt : 512 : 128],
                              op0=Alu.mult, op1=Alu.add,
                          )
                          qp = spool.tile([128, 3], f32, tag="qp")
                          dtr = spool.tile([128, 4], f32, tag="d")
                          nc.vector.scalar_tensor_tensor(
                              out=dtr[:], in0=w, scalar=1.0, in1=w,
                              op0=Alu.mult, op1=Alu.mult, accum_out=qp[:, 0:1],
                          )
                          dtr = spool.tile([128, 4], f32, tag="d")
                          nc.vector.scalar_tensor_tensor(
                              out=dtr[:], in0=w, scalar=2.0, in1=u_next[:],
                              op0=Alu.mult, op1=Alu.mult, accum_out=qp[:, 1:2],
                          )
                          dtr = spool.tile([128, 4], f32, tag="d")
                          nc.vector.scalar_tensor_tensor(
                              out=dtr[:], in0=u_next[:], scalar=1.0, in1=u_next[:],
                              op0=Alu.mult, op1=Alu.mult, accum_out=qp[:, 2:3],
                          )
                          nq_prev = ps_n.tile([128, 3], f32, tag="nq")
                          nc.tensor.matmul(nq_prev[:], ones_mat[:], qp[:],
                                           start=True, stop=True)
                      else:
                          u_next = None
                      if not fuse_z:
                          # ---- z output (raw Re(w)) on DVE ----
                          nc.vector.tensor_copy(zc[:, t : 256 : 128], wt[:, 0:2])
                      w1 = w
                      u_cur = u_next
                      a_ap = a_new

                  # ---- y^T = Wy^T @ z, scale cols by a-row, transpose, +x ----
                  y_sb = xpool.tile([128, H], f32, tag="y")
                  for hi in range(8):
                      yT_ps = ps_y.tile([128, 128], f32, tag="yT")
                      nc.tensor.matmul(
                          yT_ps[:], by_sb[:, (hi * 2 + 0) * 128 : (hi * 2 + 1) * 128],
                          zc[:, 0:128], start=True, stop=False,
                      )
                      nc.tensor.matmul(
                          yT_ps[:], by_sb[:, (hi * 2 + 1) * 128 : (hi * 2 + 2) * 128],
                          zc[:, 128:256], start=False, stop=True,
                      )
                      yTs = wpool.tile([128, 128], f32, tag="yTs")
                      nc.vector.scalar_tensor_tensor(
                          out=yTs[:], in0=yT_ps[:], scalar=1.0, in1=arow[:, 1:129],
                          op0=Alu.mult, op1=Alu.mult,
                      )
                      y_ps = ps_y.tile([128, 128], f32, tag="yT")
                      nc.tensor.matmul(y_ps[:], yTs[:], ident[:], start=True, stop=False)
                      nc.tensor.matmul(
                          y_ps[:], ident[:], x_cur[:, hi * 128 : (hi + 1) * 128],
                          start=False, stop=True,
                      )
                      nc.scalar.copy(y_sb[:, hi * 128 : (hi + 1) * 128], y_ps[:])
                  nc.sync.dma_start(yb[c * PCH : (c + 1) * PCH, :], y_sb[:])
                  x_cur, binjT_cur = x_next, binjT_next

    nc.compile()
    return nc


def _host_pack(inputs):
    """Fold all small parameters host-side; returns per-core constant arrays."""
    basis = np.asarray(inputs["basis"], np.float32)
    alpha = float(np.asarray(inputs["alpha"]))
    w_r = np.asarray(inputs["w_r"], np.float32)
    bg = _sigmoid(np.asarray(inputs["breadth_gate"], np.float32))

    g = _sigmoid(w_r)
    assert np.all(g[:MEM] == g[0]), "vector w_r gate not supported by fast path"
    gs = float(g[0])

    Wm = (basis[:, :MEM] * BETA).astype(np.float32)  # [H, 256]
    Wy = (basis[:, :MEM] * (alpha / GAMMA * bg[None, :MEM])).astype(np.float32)

    basis_m = np.concatenate(
        [Wm[hi * 128 : (hi + 1) * 128, :] for hi in range(8)], axis=1
    ).astype(np.float32)  # [128, 2048]
    WyT = np.ascontiguousarray(Wy.T)  # [256, 1024]
    blocks = []
    for hi in range(8):
        for q in range(2):
            blocks.append(WyT[q * 128 : (q + 1) * 128, hi * 128 : (hi + 1) * 128])
    basis_y = np.concatenate(blocks, axis=1).astype(np.float32)  # [128, 2048]

    t0c = (
        np.asarray(inputs["tape_init_re"], np.float32)
        + 1j * np.asarray(inputs["tape_init_im"], np.float32)
    )[:MEM].astype(np.complex64)
    nrm = np.float32(np.sqrt(max(float((np.abs(t0c) ** 2).sum(dtype=np.float32)), 1e-16)))
    v0c = (t0c / nrm).astype(np.complex64)
    v0 = np.stack(
        [v0c.real[:128], v0c.real[128:], v0c.imag[:128], v0c.imag[128:]], axis=1
    ).astype(np.float32)  # [128, 4]

    scal = np.empty((128, 3), np.float32)
    scal[:, 0] = GAMMA          # a_0 = gamma / n_{-1}, n_{-1} = 1
    scal[:, 1] = gs             # b_0 = g / n_{-2}
    scal[:, 2] = gs / GAMMA     # b_{t+1} = (g/gamma) * a_t
    return basis_m, basis_y, v0, scal


def _fast_path_ok(inputs):
    z = lambda k: np.all(np.asarray(inputs[k]) == 0)
    g = _sigmoid(np.asarray(inputs["w_r"], np.float32))
    return (
        z("torque_rotation")
        and z("epsilon_scale")
        and z("epsilon_diag")
        and z("pred_scale")
        and z("pred_diag")
        and bool(np.all(g[:MEM] == g[0]))
    )


def _numpy_fallback(inputs):
    """General-case reference implementation (host). Only used if the inputs
    violate the fast-path structure (never the case for this problem's
    generator); keeps kernel() total."""
    import jax

    with jax.default_device(jax.devices("cpu")[0]):
        import jax.numpy as jnp
        from jax import lax

        x = jnp.asarray(inputs["x"])
        basis = jnp.asarray(inputs["basis"])
        active = jnp.arange(S) < MEM
        amf = active.astype(jnp.float32)
        eta = jax.nn.softplus(jnp.asarray(inputs["eta_raw"]))
        eps = (jnp.asarray(inputs["epsilon_factor"]) * jnp.asarray(inputs["epsilon_scale"])) @ jnp.asarray(
            inputs["epsilon_factor"]).T + jnp.diag(jnp.asarray(inputs["epsilon_diag"]))
        wp = (jnp.asarray(inputs["pred_factor"]) * jnp.asarray(inputs["pred_scale"])) @ jnp.asarray(
            inputs["pred_factor"]).T + jnp.diag(jnp.asarray(inputs["pred_diag"]))
        eps_c = eps.astype(jnp.complex64)
        wp_c = wp.astype(jnp.complex64)
        rot = jnp.exp(1j * jnp.asarray(inputs["torque_rotation"]).astype(jnp.complex64))
        wr_gate = jax.nn.sigmoid(jnp.asarray(inputs["w_r"]))
        bg = jax.nn.sigmoid(jnp.asarray(inputs["breadth_gate"]))
        alpha = jnp.asarray(inputs["alpha"])

        def renorm(tape):
            masked = tape * amf
            nrm = jnp.sqrt(jnp.maximum((jnp.abs(masked) ** 2).sum(-1, keepdims=True), 1e-16))
            return masked / nrm

        tape0 = (jnp.asarray(inputs["tape_init_re"]) + 1j * jnp.asarray(inputs["tape_init_im"])) * amf
        tape0 = renorm(jnp.broadcast_to(tape0, (B, S)))

        def step(carry, x_t):
            tape, prev = carry
            m = jnp.einsum("hs,bh->bs", basis, x_t)
            mag = jnp.abs(m) * amf
            kth = lax.top_k(mag, TOPK)[0][:, -1:]
            injv = jnp.where((mag >= kth) & active, m, 0.0).astype(jnp.complex64)
            rotated = tape * rot
            drive = jnp.einsum("st,bt->bs", eps_c, rotated)
            pred = jnp.einsum("st,bt->bs", wp_c, rotated)
            new = (GAMMA * rotated + eta * drive + BETA * injv + PTS * 1j * pred + wr_gate * prev)
            new = renorm(new)
            y = x_t + alpha * jnp.einsum("hs,bs->bh", basis, bg * new.real)
            return (new, tape), y

        (_, _), ys = lax.scan(step, (tape0, tape0), jnp.swapaxes(x, 0, 1))
        return np.asarray(jnp.swapaxes(ys, 0, 1))


def kernel(n_chunks: int = T // PCH, _want_trace: bool = False,
           _root: str = "pipe", **inputs) -> np.ndarray:
    from concourse.bass_utils import run_bass_kernel_spmd

    x = np.ascontiguousarray(np.asarray(inputs["x"], np.float32))
    assert x.shape == (B, T, H)

    if not _fast_path_ok(inputs):
        return _numpy_fallback(inputs)

    basis_m, basis_y, v0, scal = _host_pack(inputs)

    key = (n_chunks, _root)
    if key not in _program_cache:
        _program_cache[key] = _build_program(n_chunks, root=_root)
    nc = _program_cache[key]

    Tq = n_chunks * PCH
    core_ids = list(range(B))
    in_maps = [
        {
            "xb": np.ascontiguousarray(x[b, :Tq]),
            "basis_m": basis_m,
            "basis_y": basis_y,
            "v0": v0,
            "scal": scal,
        }
        for b in core_ids
    ]
    res = run_bass_kernel_spmd(nc, in_maps, core_ids, trace=_want_trace)
    out = np.empty((B, Tq, H), np.float32)
    for b in core_ids:
        out[b] = res.results[b]["yb"]
    if _want_trace:
        kernel._last_results = res
    return out
